# revision 43
# baseline (speedup 1.0000x reference)
"""Causal multi-head attention block (QKV proj + SDPA + out proj) on 8 TRN2 cores.

Sharding: batch (4) x head-group (2 groups of 8 heads). Core c handles batch
c//2, heads [g*8, g*8+8) with g = c%2.

Pipelined structure: q-block (qi) outer loop. Per qi, the 4 head-pairs run
attention back-to-back with a lag-4 S->EXP->PV software pipeline so the tensor
engine never drains; QKV projection chains for the NEXT q-block are emitted as
filler between pairs; S/EXP/mask/PV all skip the fully-masked columns of
diagonal k-tiles (ragged trimming, with a single shared [128,128] triangle
mask applied via an h-folded AP). Softmax denominators come from a fused
ones-row in the PV matmul (M=65); normalization uses a scalar-engine copy of
the rowsum plus reciprocal_approx_fast on DVE, with a partition-shifted DVE
write for the second head's output half. The partial out-projection for
q-block qi is emitted after the first pair of block qi+1 (so its tensor work
fills EXP waits) and reduced across the core pair with a per-qi bf16
ReduceScatter into a per-chunk DRAM tensor (half-bias pre-folded into each
partial; fp32 conversion happens host-side).

All heavy matmuls are bf16 with fp32 PSUM accumulation. Hardcoded shapes per
the problem spec: x [4, 2048, 1024], 16 heads, head_dim 64.
"""
import sys

if '/opt/trn_rl_repo' not in sys.path:
    sys.path.insert(0, '/opt/trn_rl_repo')

import numpy as np
import ml_dtypes

import concourse.bass as bass
import concourse.mybir as mybir
from concourse import bacc
from concourse.bass_utils import run_bass_kernel_spmd
from concourse.tile import TileContext

bf16 = ml_dtypes.bfloat16
F32 = mybir.dt.float32
BF16 = mybir.dt.bfloat16

B, L, D, H, HD = 4, 2048, 1024, 16, 64
HPC = 8           # heads per core
GD = HPC * HD     # 512 dims per head-group
QB = 512          # query block
LAG = 4           # S->PV software pipeline depth (in k-tiles)

_CACHE = {}


def _build_nc():
    nc = bacc.Bacc("TRN2", target_bir_lowering=False, debug=False, num_devices=8)

    xT_d = nc.dram_tensor("xT", [D, L], BF16, kind="ExternalInput").ap()
    wT_d = nc.dram_tensor("wT", [D, 3 * GD], BF16, kind="ExternalInput").ap()
    woT_d = nc.dram_tensor("woT", [GD, D], BF16, kind="ExternalInput").ap()
    bqk_d = nc.dram_tensor("bqk", [2 * GD, 1], F32, kind="ExternalInput").ap()
    bv_d = nc.dram_tensor("bv", [128, GD], F32, kind="ExternalInput").ap()
    bob2_d = nc.dram_tensor("bob2", [128, D], F32, kind="ExternalInput").ap()
    masks_d = nc.dram_tensor("masks", [128, 256], BF16, kind="ExternalInput").ap()
    y_d = nc.dram_tensor("y", [1024, D], BF16, kind="ExternalOutput").ap()

    with TileContext(nc) as tc:
        with (
            tc.tile_pool(name="persist", bufs=1) as persist,
            tc.tile_pool(name="exps", bufs=16) as exps_pool,
            tc.tile_pool(name="pre", bufs=8) as pre_pool,
            tc.tile_pool(name="otn", bufs=2) as otn_pool,
            tc.tile_pool(name="small", bufs=4) as small,
            tc.tile_pool(name="ystage", bufs=8) as ystage,
            tc.tile_pool(name="ps_s", bufs=2, space="PSUM") as ps_s,
            tc.tile_pool(name="ps_o", bufs=1, space="PSUM") as ps_o,
            tc.tile_pool(name="ps_op", bufs=2, space="PSUM") as ps_op,
            tc.tile_pool(name="dram", bufs=1, space="DRAM") as dram,
        ):
            # ---- persistent SBUF tensors -------------------------------------
            xT = [persist.tile([128, L], BF16, tag=f"xT{c}", name=f"xT{c}") for c in range(8)]
            wT = [persist.tile([128, 3 * GD], BF16, tag=f"wT{c}", name=f"wT{c}") for c in range(8)]
            qkT = [persist.tile([128, L], BF16, tag=f"qkT{i}", name=f"qkT{i}") for i in range(8)]
            VS = HD + 1   # per-head stride in V tiles
            Vt = [persist.tile([128, HPC * VS], BF16, tag=f"V{i}", name=f"V{i}")
                  for i in range(16)]
            woT = [persist.tile([128, D], BF16, tag=f"woT{p}", name=f"woT{p}") for p in range(4)]
            bqk = persist.tile([128, 8], F32, tag="bqk")
            bv = persist.tile([128, GD], F32, tag="bv")
            bob2 = persist.tile([128, D], F32, tag="bob2")
            masks = persist.tile([128, 256], BF16, tag="masks")

            ych = [dram.tile([512, D], BF16, tag=f"ych{qi}", name=f"ych{qi}")
                   for qi in range(4)]
            yrs = [dram.tile([256, D], BF16, tag=f"yrs{qi}", name=f"yrs{qi}")
                   for qi in range(4)]

            # ---- input loads (ordered by first use) --------------------------
            for dt in range(8):
                nc.sync.dma_start(out=bqk[:, dt:dt + 1],
                                  in_=bqk_d[dt * 128:(dt + 1) * 128, :])
            for c in range(8):
                nc.sync.dma_start(out=wT[c][:, 0:512],
                                  in_=wT_d[c * 128:(c + 1) * 128, 0:512])
                nc.sync.dma_start(out=wT[c][:, 512:1024],
                                  in_=wT_d[c * 128:(c + 1) * 128, 512:1024])
                nc.sync.dma_start(out=xT[c][:, 0:512],
                                  in_=xT_d[c * 128:(c + 1) * 128, 0:512])
            for c in range(8):
                nc.sync.dma_start(out=wT[c][:, 1024:1536],
                                  in_=wT_d[c * 128:(c + 1) * 128, 1024:1536])
            nc.sync.dma_start(out=masks, in_=masks_d[:, :])
            nc.sync.dma_start(out=bv, in_=bv_d[:, :])
            for c in range(8):
                nc.sync.dma_start(out=xT[c][:, 512:1024],
                                  in_=xT_d[c * 128:(c + 1) * 128, 512:1024])
            for p in range(4):
                nc.sync.dma_start(out=woT[p], in_=woT_d[p * 128:(p + 1) * 128, :])
            nc.sync.dma_start(out=bob2, in_=bob2_d[:, :])
            for c in range(8):
                nc.sync.dma_start(out=xT[c][:, 1024:2048],
                                  in_=xT_d[c * 128:(c + 1) * 128, 1024:2048])

            # ---- projection chain emitters -----------------------------------
            def qk_chain(dt, lsb):
                ps = ps_op.tile([128, 512], F32, name="ps_proj", tag="mm")
                for c in range(8):
                    nc.tensor.matmul(
                        ps[:],
                        lhsT=wT[c][:, dt * 128:(dt + 1) * 128],
                        rhs=xT[c][:, lsb * 512:(lsb + 1) * 512],
                        start=(c == 0), stop=(c == 7),
                    )
                nc.vector.tensor_scalar_add(
                    qkT[dt][:, lsb * 512:(lsb + 1) * 512], ps[:],
                    bqk[:, dt:dt + 1],
                )

            def v_chain(lb):
                ps = ps_op.tile([128, 512], F32, name="ps_proj", tag="mm")
                for c in range(8):
                    nc.tensor.matmul(
                        ps[:],
                        lhsT=xT[c][:, lb * 128:(lb + 1) * 128],
                        rhs=wT[c][:, 1024:1536],
                        start=(c == 0), stop=(c == 7),
                    )
                v_grp = Vt[lb][:].rearrange("p (h c) -> p h c", c=VS)
                nc.vector.tensor_add(
                    v_grp[:, :, 0:HD],
                    ps[:].rearrange("p (h c) -> p h c", c=HD),
                    bv[:].rearrange("p (h c) -> p h c", c=HD),
                )
                nc.vector.memset(v_grp[:, :, HD:HD + 1], 1.0)

            def proj_block_chains(lsb, q_first=False):
                # 12 chains for this L-column block, ordered so qk lands first.
                # q_first: all Q-dim chains before K-dim ones (they only need
                # the first half of each wT tile -- cuts startup stalls).
                chains = []
                if q_first:
                    for dt in range(4):
                        chains.append(lambda dt=dt: qk_chain(dt, lsb))
                    for dt in range(4):
                        chains.append(lambda dt=dt: qk_chain(4 + dt, lsb))
                else:
                    for dt in range(4):
                        chains.append(lambda dt=dt: qk_chain(dt, lsb))
                        chains.append(lambda dt=dt: qk_chain(4 + dt, lsb))
                for i in range(4):
                    chains.append(lambda i=i: v_chain(4 * lsb + i))
                return chains

            # ---- attention pair emitter (lag-LAG pipeline) -------------------
            cur_otn = {}
            otn_hist = {}

            def emit_S(p, qi, j, pool=None):
                q0 = qi * QB
                r = j - 4 * qi
                off = 128 * r if r > 0 else 0
                ps = ps_s.tile([128, 1024], F32, name="ps_sc", tag="s")
                for hi in range(2):
                    hh = slice(hi * 64, (hi + 1) * 64)
                    nc.tensor.matmul(
                        ps[:, hi * 512 + off:(hi + 1) * 512],
                        lhsT=qkT[4 + p][hh, j * 128:(j + 1) * 128],
                        rhs=qkT[p][hh, q0 + off:q0 + 512],
                        start=True, stop=True,
                        tile_position=(64 * hi, 0),
                    )
                expt = (pool or exps_pool).tile(
                    [128, 1024], BF16,
                    tag="e" if pool is None else "pe", name="expt")
                if off == 0:
                    nc.scalar.activation(
                        expt[:], ps[:],
                        mybir.ActivationFunctionType.Exp,
                        scale=float(1.0 / np.sqrt(HD)),
                    )
                else:
                    ps3 = ps[:].rearrange("p (h q) -> p h q", h=2)
                    ex3 = expt[:].rearrange("p (h q) -> p h q", h=2)
                    nc.scalar.activation(
                        ex3[:, :, off:512], ps3[:, :, off:512],
                        mybir.ActivationFunctionType.Exp,
                        scale=float(1.0 / np.sqrt(HD)),
                    )
                if r >= 0:      # diagonal k-tile: triangle mask on 128-band
                    ex3 = expt[:].rearrange("p (h q) -> p h q", h=2)
                    m3 = masks[:].rearrange("p (h u) -> p h u", h=2)
                    nc.vector.tensor_mul(
                        ex3[:, :, off:off + 128], ex3[:, :, off:off + 128],
                        m3[:],
                    )
                return expt

            def attn_pair(p, qi, pre=None):
                js = list(range(4 * (qi + 1)))
                pso = [ps_o.tile([65, 512], F32, tag=f"o{hi}", name=f"pso{hi}")
                       for hi in range(2)]
                expts = dict(pre) if pre else {}

                def emit_PV(j):
                    r = j - 4 * qi
                    off = 128 * r if r > 0 else 0
                    for hi in range(2):
                        hl = 2 * p + hi
                        nc.tensor.matmul(
                            pso[hi][:, off:512],
                            lhsT=Vt[j][:, hl * VS:hl * VS + 65],
                            rhs=expts[j][:, hi * 512 + off:(hi + 1) * 512],
                            start=(j == js[0]), stop=(j == js[-1]),
                        )

                live = [j for j in js if j not in expts]
                for j in js:
                    if j not in live:
                        emit_PV(j)          # pre-computed expt: PV right away
                    else:
                        break
                for idx, j in enumerate(live):
                    expts[j] = emit_S(p, qi, j)
                    if idx >= LAG:
                        emit_PV(live[idx - LAG])
                for j in (live[-LAG:] if len(live) >= LAG else live):
                    emit_PV(j)

                # normalize: O^T[hd, q] / rowsum (ones row of pso)
                qsl = slice(qi * QB, (qi + 1) * QB)
                otn_t = otn_pool.tile([128, 512], BF16, tag=f"otn{p}",
                                      name=f"otn{p}")
                cur_otn[p] = otn_t
                for hi in range(2):
                    srow = small.tile([1, 512], F32, tag="srow", name="srow")
                    nc.scalar.copy(srow[:], pso[hi][64:65, :])
                    rec = small.tile([1, 512], F32, tag="rec", name="rec")
                    nc.vector.reciprocal_approx_fast(rec[:], srow[:])
                    bc = small.tile([64, 512], F32, tag="bc", name="bc")
                    nc.gpsimd.partition_broadcast(bc[:], rec[:], channels=64)
                    nc.vector.tensor_mul(
                        otn_t[hi * 64:(hi + 1) * 64, :], pso[hi][0:64, :],
                        bc[:])

            # ---- partial out-projection + chunked pair ReduceScatter ---------
            def outproj_rs(qi, subs=((0, 4),)):
                for lo, hi_ in subs:
                    for lb2 in range(lo, hi_):
                        off = lb2 * 128
                        for nh in range(2):
                            ps = ps_op.tile([128, 512], F32, name="ps_proj",
                                            tag="mm")
                            for p in range(4):
                                nc.tensor.matmul(
                                    ps[:],
                                    lhsT=otn_hist[qi][p][:, off:off + 128],
                                    rhs=woT[p][:, nh * 512:(nh + 1) * 512],
                                    start=(p == 0), stop=(p == 3),
                                )
                            yb = ystage.tile([128, 512], BF16, tag="yb",
                                             name="yb")
                            nc.vector.tensor_add(
                                yb[:], ps[:], bob2[:, nh * 512:(nh + 1) * 512])
                            nc.sync.dma_start(
                                out=ych[qi][off:off + 128,
                                            nh * 512:(nh + 1) * 512],
                                in_=yb[:],
                            )
                    rows = slice(lo * 128, hi_ * 128)
                    orows = slice(lo * 64, lo * 64 + (hi_ - lo) * 64)
                    yrows = slice(qi * 256 + lo * 64,
                                  qi * 256 + lo * 64 + (hi_ - lo) * 64)
                    nc.gpsimd.collective_compute(
                        "ReduceScatter",
                        mybir.AluOpType.add,
                        replica_groups=[[0, 1], [2, 3], [4, 5], [6, 7]],
                        ins=[ych[qi][rows, :].opt()],
                        outs=[yrs[qi][orows, :].opt()],
                    )
                    nc.sync.dma_start(out=y_d[yrows, :], in_=yrs[qi][orows, :])

            # ---- main emission schedule --------------------------------------
            for ch in proj_block_chains(0, q_first=True):
                ch()
            fillers = {qi: proj_block_chains(qi + 1) for qi in range(3)}
            pre_expts = {}
            for qi in range(4):
                for p in range(4):
                    attn_pair(p, qi, pre=pre_expts if (qi, p) == (3, 0)
                              else None)
                    if qi < 3:
                        for ch in fillers[qi][p * 3:(p + 1) * 3]:
                            ch()
                    if (qi, p) == (2, 0):
                        # pre-compute qi=3 pair-0's first 8 k-tiles (S+EXP)
                        # while the scalar engine still has slack
                        for j in range(8):
                            pre_expts[j] = emit_S(0, 3, j, pool=pre_pool)
                    if p == 3:
                        otn_hist[qi] = dict(cur_otn)
                    if p == 0 and qi > 0:
                        outproj_rs(qi - 1)
            outproj_rs(3)

    nc.compile()
    return nc


def _prep_core_inputs(c, x, Wqkv, bqkv, Wo, bo, masks_np):
    b, g = c // 2, c % 2
    qs = slice(g * GD, (g + 1) * GD)
    ks = slice(D + g * GD, D + (g + 1) * GD)
    vs = slice(2 * D + g * GD, 2 * D + (g + 1) * GD)
    Wc = np.concatenate([Wqkv[qs], Wqkv[ks], Wqkv[vs]], axis=0)
    return {
        "xT": np.ascontiguousarray(x[b].T).astype(bf16),
        "wT": np.ascontiguousarray(Wc.T).astype(bf16),
        "woT": np.ascontiguousarray(Wo[:, g * GD:(g + 1) * GD].T).astype(bf16),
        "bqk": np.concatenate([bqkv[qs], bqkv[ks]]).astype(np.float32).reshape(2 * GD, 1),
        "bv": np.tile(bqkv[vs].astype(np.float32), (128, 1)),
        "bob2": np.tile(0.5 * bo.astype(np.float32), (128, 1)),
        "masks": masks_np,
    }


def _masks_np():
    kk = np.arange(128)[:, None]
    qq = np.arange(128)[None, :]
    tri = (qq >= kk).astype(bf16)
    return np.tile(tri, (1, 2))


def _run(inputs, trace=False):
    if "nc" not in _CACHE:
        _CACHE["nc"] = _build_nc()
    nc = _CACHE["nc"]
    x = np.asarray(inputs["x"], dtype=np.float32)
    Wqkv = np.asarray(inputs["Wqkv"], dtype=np.float32)
    bqkv = np.asarray(inputs["bqkv"], dtype=np.float32)
    Wo = np.asarray(inputs["Wo"], dtype=np.float32)
    bo = np.asarray(inputs["bo"], dtype=np.float32)
    masks_np = _masks_np()
    in_maps = [_prep_core_inputs(c, x, Wqkv, bqkv, Wo, bo, masks_np)
               for c in range(8)]
    res = run_bass_kernel_spmd(nc, in_maps, core_ids=list(range(8)), trace=trace)
    out = np.empty((B, L, D), dtype=np.float32)
    for b in range(B):
        for g in range(2):
            yc = res.results[2 * b + g]["y"]
            for qi in range(4):
                dst = qi * 512 + g * 256
                src_ = qi * 256
                out[b, dst:dst + 256] = yc[src_:src_ + 256]
    return out, res


def kernel(x, mask, Wqkv, bqkv, Wo, bo):
    out, _ = _run({"x": x, "mask": mask, "Wqkv": Wqkv, "bqkv": bqkv,
                   "Wo": Wo, "bo": bo})
    return out


def kernel_traced(x, mask, Wqkv, bqkv, Wo, bo):
    return _run({"x": x, "mask": mask, "Wqkv": Wqkv, "bqkv": bqkv,
                 "Wo": Wo, "bo": bo}, trace=True)


# revision 44
# speedup vs baseline: 1.0175x; 1.0175x over previous
"""Causal multi-head attention block (QKV proj + SDPA + out proj) on 8 TRN2 cores.

Sharding: batch (4) x head-group (2 groups of 8 heads). Core c handles batch
c//2, heads [g*8, g*8+8) with g = c%2.

Pipelined structure: q-block (qi) outer loop. Per qi, the 4 head-pairs run
attention back-to-back with a lag-4 S->EXP->PV software pipeline so the tensor
engine never drains; QKV projection chains for the NEXT q-block are emitted as
filler between pairs; S/EXP/mask/PV all skip the fully-masked columns of
diagonal k-tiles (ragged trimming, with a single shared [128,128] triangle
mask applied via an h-folded AP). Softmax denominators come from a fused
ones-row in the PV matmul (M=65); normalization uses a scalar-engine copy of
the rowsum plus reciprocal_approx_fast on DVE, with a partition-shifted DVE
write for the second head's output half. The partial out-projection for
q-block qi is emitted after the first pair of block qi+1 (so its tensor work
fills EXP waits) and reduced across the core pair with a per-qi bf16
ReduceScatter into a per-chunk DRAM tensor (half-bias pre-folded into each
partial; fp32 conversion happens host-side).

All heavy matmuls are bf16 with fp32 PSUM accumulation. Hardcoded shapes per
the problem spec: x [4, 2048, 1024], 16 heads, head_dim 64.
"""
import sys

if '/opt/trn_rl_repo' not in sys.path:
    sys.path.insert(0, '/opt/trn_rl_repo')

import numpy as np
import ml_dtypes

import concourse.bass as bass
import concourse.mybir as mybir
from concourse import bacc
from concourse.bass_utils import run_bass_kernel_spmd
from concourse.tile import TileContext

bf16 = ml_dtypes.bfloat16
F32 = mybir.dt.float32
BF16 = mybir.dt.bfloat16

B, L, D, H, HD = 4, 2048, 1024, 16, 64
HPC = 8           # heads per core
GD = HPC * HD     # 512 dims per head-group
QB = 512          # query block
LAG = 4           # S->PV software pipeline depth (in k-tiles)

_CACHE = {}


def _build_nc():
    nc = bacc.Bacc("TRN2", target_bir_lowering=False, debug=False, num_devices=8)

    xT_d = nc.dram_tensor("xT", [D, L], BF16, kind="ExternalInput").ap()
    wT_d = nc.dram_tensor("wT", [D, 3 * GD], BF16, kind="ExternalInput").ap()
    woT_d = nc.dram_tensor("woT", [GD, D], BF16, kind="ExternalInput").ap()
    bqk_d = nc.dram_tensor("bqk", [2 * GD, 1], F32, kind="ExternalInput").ap()
    bv_d = nc.dram_tensor("bv", [128, GD], F32, kind="ExternalInput").ap()
    bob2_d = nc.dram_tensor("bob2", [128, D], F32, kind="ExternalInput").ap()
    masks_d = nc.dram_tensor("masks", [128, 256], BF16, kind="ExternalInput").ap()
    y_d = nc.dram_tensor("y", [1024, D], BF16, kind="ExternalOutput").ap()

    with TileContext(nc) as tc:
        with (
            tc.tile_pool(name="persist", bufs=1) as persist,
            tc.tile_pool(name="exps", bufs=16) as exps_pool,
            tc.tile_pool(name="pre", bufs=8) as pre_pool,
            tc.tile_pool(name="otn", bufs=2) as otn_pool,
            tc.tile_pool(name="small", bufs=3) as small,
            tc.tile_pool(name="ystage", bufs=8) as ystage,
            tc.tile_pool(name="ps_s", bufs=2, space="PSUM") as ps_s,
            tc.tile_pool(name="ps_o", bufs=1, space="PSUM") as ps_o,
            tc.tile_pool(name="ps_op", bufs=2, space="PSUM") as ps_op,
            tc.tile_pool(name="dram", bufs=1, space="DRAM") as dram,
        ):
            # ---- persistent SBUF tensors -------------------------------------
            xT = [persist.tile([128, L], BF16, tag=f"xT{c}", name=f"xT{c}") for c in range(8)]
            wT = [persist.tile([128, 3 * GD], BF16, tag=f"wT{c}", name=f"wT{c}") for c in range(8)]
            qkT = [persist.tile([128, L], BF16, tag=f"qkT{i}", name=f"qkT{i}") for i in range(8)]
            VS = HD + 1   # per-head stride in V tiles
            Vt = [persist.tile([128, HPC * VS], BF16, tag=f"V{i}", name=f"V{i}")
                  for i in range(16)]
            woT = [persist.tile([128, D], BF16, tag=f"woT{p}", name=f"woT{p}") for p in range(4)]
            bqk = persist.tile([128, 8], F32, tag="bqk")
            bv = persist.tile([128, GD], F32, tag="bv")
            bob2 = persist.tile([128, D], F32, tag="bob2")
            masks = persist.tile([128, 256], BF16, tag="masks")

            ych = [dram.tile([512, D], BF16, tag=f"ych{qi}", name=f"ych{qi}")
                   for qi in range(4)]
            yrs = [dram.tile([256, D], BF16, tag=f"yrs{qi}", name=f"yrs{qi}")
                   for qi in range(4)]

            # ---- input loads (ordered by first use) --------------------------
            for dt in range(8):
                nc.sync.dma_start(out=bqk[:, dt:dt + 1],
                                  in_=bqk_d[dt * 128:(dt + 1) * 128, :])
            for c in range(8):
                nc.sync.dma_start(out=wT[c][:, 0:512],
                                  in_=wT_d[c * 128:(c + 1) * 128, 0:512])
                nc.sync.dma_start(out=wT[c][:, 512:1024],
                                  in_=wT_d[c * 128:(c + 1) * 128, 512:1024])
                nc.sync.dma_start(out=xT[c][:, 0:512],
                                  in_=xT_d[c * 128:(c + 1) * 128, 0:512])
            for c in range(8):
                nc.sync.dma_start(out=wT[c][:, 1024:1536],
                                  in_=wT_d[c * 128:(c + 1) * 128, 1024:1536])
            nc.sync.dma_start(out=masks, in_=masks_d[:, :])
            nc.sync.dma_start(out=bv, in_=bv_d[:, :])
            for c in range(8):
                nc.sync.dma_start(out=xT[c][:, 512:1024],
                                  in_=xT_d[c * 128:(c + 1) * 128, 512:1024])
            for p in range(4):
                nc.sync.dma_start(out=woT[p], in_=woT_d[p * 128:(p + 1) * 128, :])
            nc.sync.dma_start(out=bob2, in_=bob2_d[:, :])
            for c in range(8):
                nc.sync.dma_start(out=xT[c][:, 1024:2048],
                                  in_=xT_d[c * 128:(c + 1) * 128, 1024:2048])

            # ---- projection chain emitters -----------------------------------
            def qk_chain(dt, lsb):
                ps = ps_op.tile([128, 512], F32, name="ps_proj", tag="mm")
                for c in range(8):
                    nc.tensor.matmul(
                        ps[:],
                        lhsT=wT[c][:, dt * 128:(dt + 1) * 128],
                        rhs=xT[c][:, lsb * 512:(lsb + 1) * 512],
                        start=(c == 0), stop=(c == 7),
                    )
                nc.vector.tensor_scalar_add(
                    qkT[dt][:, lsb * 512:(lsb + 1) * 512], ps[:],
                    bqk[:, dt:dt + 1],
                )

            def v_chain(lb):
                ps = ps_op.tile([128, 512], F32, name="ps_proj", tag="mm")
                for c in range(8):
                    nc.tensor.matmul(
                        ps[:],
                        lhsT=xT[c][:, lb * 128:(lb + 1) * 128],
                        rhs=wT[c][:, 1024:1536],
                        start=(c == 0), stop=(c == 7),
                    )
                v_grp = Vt[lb][:].rearrange("p (h c) -> p h c", c=VS)
                nc.vector.tensor_add(
                    v_grp[:, :, 0:HD],
                    ps[:].rearrange("p (h c) -> p h c", c=HD),
                    bv[:].rearrange("p (h c) -> p h c", c=HD),
                )
                nc.vector.memset(v_grp[:, :, HD:HD + 1], 1.0)

            def proj_block_chains(lsb, q_first=False):
                # 12 chains for this L-column block, ordered so qk lands first.
                # q_first: all Q-dim chains before K-dim ones (they only need
                # the first half of each wT tile -- cuts startup stalls).
                chains = []
                if q_first:
                    for dt in range(4):
                        chains.append(lambda dt=dt: qk_chain(dt, lsb))
                    for dt in range(4):
                        chains.append(lambda dt=dt: qk_chain(4 + dt, lsb))
                else:
                    for dt in range(4):
                        chains.append(lambda dt=dt: qk_chain(dt, lsb))
                        chains.append(lambda dt=dt: qk_chain(4 + dt, lsb))
                for i in range(4):
                    chains.append(lambda i=i: v_chain(4 * lsb + i))
                return chains

            # ---- attention pair emitter (lag-LAG pipeline) -------------------
            cur_otn = {}
            otn_hist = {}

            def emit_S(p, qi, j, pool=None):
                q0 = qi * QB
                r = j - 4 * qi
                off = 128 * r if r > 0 else 0
                ps = ps_s.tile([128, 1024], F32, name="ps_sc", tag="s")
                for hi in range(2):
                    hh = slice(hi * 64, (hi + 1) * 64)
                    nc.tensor.matmul(
                        ps[:, hi * 512 + off:(hi + 1) * 512],
                        lhsT=qkT[4 + p][hh, j * 128:(j + 1) * 128],
                        rhs=qkT[p][hh, q0 + off:q0 + 512],
                        start=True, stop=True,
                        tile_position=(64 * hi, 0),
                    )
                expt = (pool or exps_pool).tile(
                    [128, 1024], BF16,
                    tag="e" if pool is None else "pe", name="expt")
                if off == 0:
                    nc.scalar.activation(
                        expt[:], ps[:],
                        mybir.ActivationFunctionType.Exp,
                        scale=float(1.0 / np.sqrt(HD)),
                    )
                else:
                    ps3 = ps[:].rearrange("p (h q) -> p h q", h=2)
                    ex3 = expt[:].rearrange("p (h q) -> p h q", h=2)
                    nc.scalar.activation(
                        ex3[:, :, off:512], ps3[:, :, off:512],
                        mybir.ActivationFunctionType.Exp,
                        scale=float(1.0 / np.sqrt(HD)),
                    )
                if r >= 0:      # diagonal k-tile: triangle mask on 128-band
                    ex3 = expt[:].rearrange("p (h q) -> p h q", h=2)
                    m3 = masks[:].rearrange("p (h u) -> p h u", h=2)
                    nc.vector.tensor_mul(
                        ex3[:, :, off:off + 128], ex3[:, :, off:off + 128],
                        m3[:],
                    )
                return expt

            def attn_pair(p, qi, pre=None):
                js = list(range(4 * (qi + 1)))
                pso = [ps_o.tile([65, 512], F32, tag=f"o{hi}", name=f"pso{hi}")
                       for hi in range(2)]
                expts = dict(pre) if pre else {}

                def emit_PV(j):
                    r = j - 4 * qi
                    off = 128 * r if r > 0 else 0
                    for hi in range(2):
                        hl = 2 * p + hi
                        nc.tensor.matmul(
                            pso[hi][:, off:512],
                            lhsT=Vt[j][:, hl * VS:hl * VS + 65],
                            rhs=expts[j][:, hi * 512 + off:(hi + 1) * 512],
                            start=(j == js[0]), stop=(j == js[-1]),
                        )

                live = [j for j in js if j not in expts]
                for j in js:
                    if j not in live:
                        emit_PV(j)          # pre-computed expt: PV right away
                    else:
                        break
                for idx, j in enumerate(live):
                    expts[j] = emit_S(p, qi, j)
                    if idx >= LAG:
                        emit_PV(live[idx - LAG])
                for j in (live[-LAG:] if len(live) >= LAG else live):
                    emit_PV(j)

                # normalize: O^T[hd, q] / rowsum (ones row of pso)
                qsl = slice(qi * QB, (qi + 1) * QB)
                otn_t = otn_pool.tile([128, 512], BF16, tag=f"otn{p}",
                                      name=f"otn{p}")
                cur_otn[p] = otn_t
                for hi in range(2):
                    srow = small.tile([1, 512], F32, tag="srow", name="srow")
                    nc.scalar.copy(srow[:], pso[hi][64:65, :])
                    rec = small.tile([1, 512], F32, tag="rec", name="rec")
                    nc.vector.reciprocal_approx_fast(rec[:], srow[:])
                    bc = small.tile([64, 512], F32, tag="bc", name="bc")
                    nc.gpsimd.partition_broadcast(bc[:], rec[:], channels=64)
                    nc.vector.tensor_mul(
                        otn_t[hi * 64:(hi + 1) * 64, :], pso[hi][0:64, :],
                        bc[:])

            # ---- partial out-projection + chunked pair ReduceScatter ---------
            def outproj_rs(qi, subs=((0, 4),)):
                for lo, hi_ in subs:
                    for lb2 in range(lo, hi_):
                        off = lb2 * 128
                        for nh in range(2):
                            ps = ps_op.tile([128, 512], F32, name="ps_proj",
                                            tag="mm")
                            for p in range(4):
                                nc.tensor.matmul(
                                    ps[:],
                                    lhsT=otn_hist[qi][p][:, off:off + 128],
                                    rhs=woT[p][:, nh * 512:(nh + 1) * 512],
                                    start=(p == 0), stop=(p == 3),
                                )
                            yb = ystage.tile([128, 512], BF16, tag="yb",
                                             name="yb")
                            nc.vector.tensor_add(
                                yb[:], ps[:], bob2[:, nh * 512:(nh + 1) * 512])
                            nc.sync.dma_start(
                                out=ych[qi][off:off + 128,
                                            nh * 512:(nh + 1) * 512],
                                in_=yb[:],
                            )
                    rows = slice(lo * 128, hi_ * 128)
                    orows = slice(lo * 64, lo * 64 + (hi_ - lo) * 64)
                    yrows = slice(qi * 256 + lo * 64,
                                  qi * 256 + lo * 64 + (hi_ - lo) * 64)
                    nc.gpsimd.collective_compute(
                        "ReduceScatter",
                        mybir.AluOpType.add,
                        replica_groups=[[0, 1], [2, 3], [4, 5], [6, 7]],
                        ins=[ych[qi][rows, :].opt()],
                        outs=[yrs[qi][orows, :].opt()],
                    )
                    nc.sync.dma_start(out=y_d[yrows, :], in_=yrs[qi][orows, :])

            # ---- main emission schedule --------------------------------------
            for ch in proj_block_chains(0, q_first=True):
                ch()
            fillers = {qi: proj_block_chains(qi + 1) for qi in range(3)}
            pre_expts = {}
            for qi in range(4):
                for p in range(4):
                    attn_pair(p, qi, pre=pre_expts if (qi, p) == (3, 0)
                              else None)
                    if qi < 3:
                        for ch in fillers[qi][p * 3:(p + 1) * 3]:
                            ch()
                    if (qi, p) == (2, 0):
                        # pre-compute qi=3 pair-0's first 8 k-tiles (S+EXP)
                        # while the scalar engine still has slack
                        for j in range(8):
                            pre_expts[j] = emit_S(0, 3, j, pool=pre_pool)
                    if p == 3:
                        otn_hist[qi] = dict(cur_otn)
                    if p == 0 and qi > 0:
                        outproj_rs(qi - 1)
            outproj_rs(3)

    nc.compile()
    return nc


def _prep_core_inputs(c, x, Wqkv, bqkv, Wo, bo, masks_np):
    b, g = c // 2, c % 2
    qs = slice(g * GD, (g + 1) * GD)
    ks = slice(D + g * GD, D + (g + 1) * GD)
    vs = slice(2 * D + g * GD, 2 * D + (g + 1) * GD)
    Wc = np.concatenate([Wqkv[qs], Wqkv[ks], Wqkv[vs]], axis=0)
    return {
        "xT": np.ascontiguousarray(x[b].T).astype(bf16),
        "wT": np.ascontiguousarray(Wc.T).astype(bf16),
        "woT": np.ascontiguousarray(Wo[:, g * GD:(g + 1) * GD].T).astype(bf16),
        "bqk": np.concatenate([bqkv[qs], bqkv[ks]]).astype(np.float32).reshape(2 * GD, 1),
        "bv": np.tile(bqkv[vs].astype(np.float32), (128, 1)),
        "bob2": np.tile(0.5 * bo.astype(np.float32), (128, 1)),
        "masks": masks_np,
    }


def _masks_np():
    kk = np.arange(128)[:, None]
    qq = np.arange(128)[None, :]
    tri = (qq >= kk).astype(bf16)
    return np.tile(tri, (1, 2))


def _run(inputs, trace=False):
    if "nc" not in _CACHE:
        _CACHE["nc"] = _build_nc()
    nc = _CACHE["nc"]
    x = np.asarray(inputs["x"], dtype=np.float32)
    Wqkv = np.asarray(inputs["Wqkv"], dtype=np.float32)
    bqkv = np.asarray(inputs["bqkv"], dtype=np.float32)
    Wo = np.asarray(inputs["Wo"], dtype=np.float32)
    bo = np.asarray(inputs["bo"], dtype=np.float32)
    masks_np = _masks_np()
    in_maps = [_prep_core_inputs(c, x, Wqkv, bqkv, Wo, bo, masks_np)
               for c in range(8)]
    res = run_bass_kernel_spmd(nc, in_maps, core_ids=list(range(8)), trace=trace)
    out = np.empty((B, L, D), dtype=np.float32)
    for b in range(B):
        for g in range(2):
            yc = res.results[2 * b + g]["y"]
            for qi in range(4):
                dst = qi * 512 + g * 256
                src_ = qi * 256
                out[b, dst:dst + 256] = yc[src_:src_ + 256]
    return out, res


def kernel(x, mask, Wqkv, bqkv, Wo, bo):
    out, _ = _run({"x": x, "mask": mask, "Wqkv": Wqkv, "bqkv": bqkv,
                   "Wo": Wo, "bo": bo})
    return out


def kernel_traced(x, mask, Wqkv, bqkv, Wo, bo):
    return _run({"x": x, "mask": mask, "Wqkv": Wqkv, "bqkv": bqkv,
                 "Wo": Wo, "bo": bo}, trace=True)
